# revision 4
# baseline (speedup 1.0000x reference)
"""Adaptive memory update kernel for 8 Trainium2 NeuronCores.

Reference computation (B=4096, D=1024, N_VIDEOS=100000):
    alpha      = sigmoid(h_last @ W_alpha + b_alpha)          # [B, 1]
    M          = mem[vids]                                     # [B, D]
    M_new      = alpha * M + (1 - alpha) * h_last
    M_smoothed = d * M + (1 - d) * M_new                       # d = medium_decay
    return M_smoothed                                          # [B, D]

Algebra used on device:  out = beta * h + gamma * M   with
    beta  = (1 - d) * (1 - alpha) = (1 - d) * sigmoid(-(h@W + b))
    gamma = 1 - beta

Sharding: data-parallel over the batch. Core i gets rows [512*i, 512*(i+1))
of h_last, and the memory rows for those vids are routed to it on the host
(host gather = the "route each row to the device owning that vid" step).
Each core streams 4 tiles of [128, 1024]:
    DVE   : tensor_tensor_reduce  -> xneg = -(h.W + b)   (fused mul+reduce)
    ACT   : sigmoid(xneg) -> s;  t = gamma * M           (Copy with AP scale)
    DVE   : beta = s * (1-d); gamma = 1 - beta           ([128,1] ops)
    GPSIMD: out = (h * beta) + t                         (fused scalar_tensor_tensor)
"""

import numpy as np

B = 4096
D = 1024
N_CORES = 8
ROWS = B // N_CORES  # 512 rows per core
P = 128              # SBUF partitions
G = ROWS // P        # 4 tiles per core

_CACHE: dict = {}


def _build():
    if "nc" in _CACHE:
        return _CACHE["nc"]

    import concourse.bass as bass
    import concourse.tile as tile
    from concourse import bacc, mybir

    f32 = mybir.dt.float32
    Alu = mybir.AluOpType
    Act = mybir.ActivationFunctionType

    nc = bacc.Bacc("TRN2", target_bir_lowering=False, debug=False,
                   num_devices=N_CORES)

    h_ext = nc.dram_tensor("h", [ROWS, D], f32, kind="ExternalInput").ap()
    m_ext = nc.dram_tensor("m", [ROWS, D], f32, kind="ExternalInput").ap()
    wb_ext = nc.dram_tensor("wb", [P, D], f32, kind="ExternalInput").ap()
    # aux[:, 0] = -b_alpha, aux[:, 1] = 1 - medium_decay (broadcast per partition)
    aux_ext = nc.dram_tensor("aux", [P, 2], f32, kind="ExternalInput").ap()
    out_ext = nc.dram_tensor("out", [ROWS, D], f32, kind="ExternalOutput").ap()

    with tile.TileContext(nc) as tc:
        with tc.tile_pool(name="const", bufs=1) as cpool, \
             tc.tile_pool(name="io", bufs=3) as io, \
             tc.tile_pool(name="tmp", bufs=3) as tmp, \
             tc.tile_pool(name="vec", bufs=8) as vec:
            wb = cpool.tile([P, D], f32)
            nc.sync.dma_start(wb[:], wb_ext[:, :])
            aux = cpool.tile([P, 2], f32)
            nc.sync.dma_start(aux[:], aux_ext[:, :])
            nb = aux[:, 0:1]   # -b_alpha
            nd = aux[:, 1:2]   # 1 - d

            for g in range(G):
                ht = io.tile([P, D], f32, tag="h")
                nc.sync.dma_start(ht[:], h_ext[bass.ts(g, P), :])
                mt = io.tile([P, D], f32, tag="m")
                nc.sync.dma_start(mt[:], m_ext[bass.ts(g, P), :])

                # x = h.W (fused mul + row-sum on DVE)
                scratch = tmp.tile([P, D], f32, tag="scratch")
                x = vec.tile([P, 1], f32, tag="x")
                nc.vector.scalar_tensor_tensor(
                    out=scratch[:], in0=ht[:], scalar=1.0, in1=wb[:],
                    op0=Alu.mult, op1=Alu.mult,
                    accum_out=x[:],
                )
                # s = sigmoid(-(x + b)) = 1 - alpha   (bias AP nb = -b)
                s = vec.tile([P, 1], f32, tag="s")
                nc.scalar.activation(s[:], x[:], Act.Sigmoid,
                                     bias=nb, scale=-1.0)
                # beta = (1-d) * s ; gamma = 1 - beta
                beta = vec.tile([P, 1], f32, tag="beta")
                nc.vector.tensor_scalar_mul(beta[:], s[:], nd)
                gamma = vec.tile([P, 1], f32, tag="gamma")
                nc.vector.tensor_scalar(
                    out=gamma[:], in0=beta[:],
                    scalar1=-1.0, scalar2=1.0,
                    op0=Alu.mult, op1=Alu.add,
                )
                # t = gamma * M  (ScalarE Copy with per-partition scale)
                t = tmp.tile([P, D], f32, tag="t")
                nc.scalar.mul(t[:], mt[:], gamma[:])
                # out = (h * beta) + t  (DVE fused)
                o = tmp.tile([P, D], f32, tag="o")
                nc.vector.scalar_tensor_tensor(
                    out=o[:], in0=ht[:], scalar=beta[:], in1=t[:],
                    op0=Alu.mult, op1=Alu.add,
                )
                nc.sync.dma_start(out_ext[bass.ts(g, P), :], o[:])

    nc.compile()
    _CACHE["nc"] = nc
    return nc


def kernel(h_last, vids, mem, W_alpha, b_alpha, medium_decay, **run_kwargs):
    from concourse.bass_utils import run_bass_kernel_spmd

    h = np.ascontiguousarray(np.asarray(h_last, dtype=np.float32))
    v = np.asarray(vids).astype(np.int64, copy=False)
    mem = np.asarray(mem, dtype=np.float32)
    m_rows = np.ascontiguousarray(mem[v])  # host routing: gather owned rows
    w = np.asarray(W_alpha, dtype=np.float32).reshape(D)
    wb = np.ascontiguousarray(np.broadcast_to(w[None, :], (P, D)))
    b = float(np.asarray(b_alpha, dtype=np.float32).reshape(-1)[0])
    d = float(np.asarray(medium_decay, dtype=np.float32))
    aux = np.empty((P, 2), dtype=np.float32)
    aux[:, 0] = -b
    aux[:, 1] = 1.0 - d

    nc = _build()
    in_maps = []
    for c in range(N_CORES):
        sl = slice(c * ROWS, (c + 1) * ROWS)
        in_maps.append({"h": h[sl], "m": m_rows[sl], "wb": wb, "aux": aux})

    res = run_bass_kernel_spmd(nc, in_maps, core_ids=list(range(N_CORES)),
                               **run_kwargs)
    _CACHE["_last_res"] = res
    out = np.concatenate([res.results[c]["out"] for c in range(N_CORES)], axis=0)
    return np.ascontiguousarray(out, dtype=np.float32)


# revision 6
# speedup vs baseline: 1.1223x; 1.1223x over previous
"""Adaptive memory update kernel for 8 Trainium2 NeuronCores.

Reference computation (B=4096, D=1024, N_VIDEOS=100000):
    alpha      = sigmoid(h_last @ W_alpha + b_alpha)          # [B, 1]
    M          = mem[vids]                                     # [B, D]
    M_new      = alpha * M + (1 - alpha) * h_last
    M_smoothed = d * M + (1 - d) * M_new                       # d = medium_decay
    return M_smoothed                                          # [B, D]

Algebra used on device:  out = beta * h + gamma * M   with
    beta  = (1 - d) * (1 - alpha) = (1 - d) * sigmoid(-(h@W + b))
    gamma = 1 - beta

Sharding: data-parallel over the batch. Core i gets rows [512*i, 512*(i+1))
of h_last, and the memory rows for those vids are routed to it on the host
(host gather = the "route each row to the device owning that vid" step).

Device kernel (per core, bf16 I/O for 2x DMA saving; tolerance is 2e-2,
bf16 rounding contributes ~4e-3):
  - stream [128, 2*D] tiles (two 128-row groups per DMA, 0.5 MB transfers)
  - DVE   : scalar_tensor_tensor (h*1)*W with accum_out -> x = h.W per row
  - ACT   : s = sigmoid(-x - b);  t = gamma * M  (Copy with AP scale)
  - DVE   : beta = s*(1-d); gamma = 1-beta ([128,1]); out = (h*beta)+t (fused)
"""

import numpy as np

B = 4096
D = 1024
N_CORES = 8
ROWS = B // N_CORES  # 512 rows per core
P = 128              # SBUF partitions
G = ROWS // P        # 4 row-groups per core
GPT = 2              # row-groups per DMA tile
NT = G // GPT        # DMA tiles per core

_CACHE: dict = {}


def _build(use_bf16: bool = True):
    key = ("nc", use_bf16)
    if key in _CACHE:
        return _CACHE[key]

    import concourse.bass as bass
    import concourse.tile as tile
    from concourse import bacc, mybir

    f32 = mybir.dt.float32
    dt_io = mybir.dt.bfloat16 if use_bf16 else f32
    Alu = mybir.AluOpType
    Act = mybir.ActivationFunctionType
    W_FREE = GPT * D  # free size of one DMA tile

    nc = bacc.Bacc("TRN2", target_bir_lowering=False, debug=False,
                   num_devices=N_CORES)

    h_ext = nc.dram_tensor("h", [ROWS, D], dt_io, kind="ExternalInput").ap()
    m_ext = nc.dram_tensor("m", [ROWS, D], dt_io, kind="ExternalInput").ap()
    wb_ext = nc.dram_tensor("wb", [P, D], dt_io, kind="ExternalInput").ap()
    # aux[:, 0] = -b_alpha, aux[:, 1] = 1 - medium_decay (broadcast per partition)
    aux_ext = nc.dram_tensor("aux", [P, 2], f32, kind="ExternalInput").ap()
    out_ext = nc.dram_tensor("out", [ROWS, D], dt_io, kind="ExternalOutput").ap()

    # [ROWS, D] viewed as [128, NT * GPT * D]: tile t, block b holds rows
    # [t*GPT*P + b*P + p], cols = D contiguous
    h_r = h_ext.rearrange("(t b p) d -> p t b d", p=P, b=GPT)
    m_r = m_ext.rearrange("(t b p) d -> p t b d", p=P, b=GPT)
    o_r = out_ext.rearrange("(t b p) d -> p t b d", p=P, b=GPT)

    with tile.TileContext(nc) as tc:
        with tc.tile_pool(name="const", bufs=1) as cpool, \
             tc.tile_pool(name="io", bufs=3) as io, \
             tc.tile_pool(name="tmp", bufs=3) as tmp, \
             tc.tile_pool(name="vec", bufs=8) as vec:
            wb = cpool.tile([P, D], dt_io)
            nc.sync.dma_start(wb[:], wb_ext[:, :])
            aux = cpool.tile([P, 2], f32)
            nc.sync.dma_start(aux[:], aux_ext[:, :])
            nb = aux[:, 0:1]   # -b_alpha
            nd = aux[:, 1:2]   # 1 - d

            for t in range(NT):
                ht = io.tile([P, GPT, D], dt_io, tag="h")
                nc.sync.dma_start(ht[:], h_r[:, t])
                mt = io.tile([P, GPT, D], dt_io, tag="m")
                nc.sync.dma_start(mt[:], m_r[:, t])

                o = tmp.tile([P, GPT, D], dt_io, tag="o")
                scratch = tmp.tile([P, D], dt_io, tag="scratch")
                for bk in range(GPT):
                    # x = h.W (fused mul + row-sum on DVE)
                    x = vec.tile([P, 1], f32, tag="x")
                    nc.vector.scalar_tensor_tensor(
                        out=scratch[:], in0=ht[:, bk], scalar=1.0,
                        in1=wb[:], op0=Alu.mult, op1=Alu.mult,
                        accum_out=x[:],
                    )
                    # s = sigmoid(-(x + b)) = 1 - alpha   (bias AP nb = -b)
                    s = vec.tile([P, 1], f32, tag="s")
                    nc.scalar.activation(s[:], x[:], Act.Sigmoid,
                                         bias=nb, scale=-1.0)
                    # beta = (1-d) * s ; gamma = 1 - beta
                    beta = vec.tile([P, 1], f32, tag="beta")
                    nc.vector.tensor_scalar_mul(beta[:], s[:], nd)
                    gamma = vec.tile([P, 1], f32, tag="gamma")
                    nc.vector.tensor_scalar(
                        out=gamma[:], in0=beta[:],
                        scalar1=-1.0, scalar2=1.0,
                        op0=Alu.mult, op1=Alu.add,
                    )
                    # t = gamma * M  (ScalarE Copy with per-partition scale)
                    tg = tmp.tile([P, D], dt_io, tag="t")
                    nc.scalar.mul(tg[:], mt[:, bk], gamma[:])
                    # out = (h * beta) + t  (DVE fused)
                    nc.vector.scalar_tensor_tensor(
                        out=o[:, bk], in0=ht[:, bk], scalar=beta[:],
                        in1=tg[:], op0=Alu.mult, op1=Alu.add,
                    )
                nc.sync.dma_start(o_r[:, t], o[:])

    nc.compile()
    _CACHE[key] = nc
    return nc


def kernel(h_last, vids, mem, W_alpha, b_alpha, medium_decay,
           use_bf16: bool = True, **run_kwargs):
    import ml_dtypes
    from concourse.bass_utils import run_bass_kernel_spmd

    np_io = ml_dtypes.bfloat16 if use_bf16 else np.float32

    h = np.ascontiguousarray(np.asarray(h_last, dtype=np.float32).astype(np_io))
    v = np.asarray(vids).astype(np.int64, copy=False)
    mem = np.asarray(mem, dtype=np.float32)
    m_rows = np.ascontiguousarray(mem[v].astype(np_io))  # host routing
    w = np.asarray(W_alpha, dtype=np.float32).reshape(D)
    wb = np.ascontiguousarray(np.broadcast_to(w[None, :], (P, D)).astype(np_io))
    b = float(np.asarray(b_alpha, dtype=np.float32).reshape(-1)[0])
    d = float(np.asarray(medium_decay, dtype=np.float32))
    aux = np.empty((P, 2), dtype=np.float32)
    aux[:, 0] = -b
    aux[:, 1] = 1.0 - d

    nc = _build(use_bf16)
    in_maps = []
    for c in range(N_CORES):
        sl = slice(c * ROWS, (c + 1) * ROWS)
        in_maps.append({"h": h[sl], "m": m_rows[sl], "wb": wb, "aux": aux})

    res = run_bass_kernel_spmd(nc, in_maps, core_ids=list(range(N_CORES)),
                               **run_kwargs)
    _CACHE["_last_res"] = res
    out = np.concatenate([res.results[c]["out"] for c in range(N_CORES)], axis=0)
    return np.ascontiguousarray(out.astype(np.float32))
